# revision 7
# baseline (speedup 1.0000x reference)
"""DeepReservoir (2-layer leaky ESN) Trainium2 kernel, v2.

Reference computation (per layer):
    u = x @ K + b
    h_t = 0.1*h_{t-1} + 0.9*tanh(u_t + h_{t-1} @ W)
Layer 1 consumes layer 0's states; output = concat(s0, s1) on features.

Kernel strategy (data-parallel over batch, 8 cores x B=4 samples):
  - On-chip layout: features on partitions; state is step-major
    [128 part, t*16 + j*4 + b] (j = feature 128-tile, b = sample), so the
    per-step state slice is a contiguous [128,16] block.
  - u chunks are precomputed by per-chunk GEMMs DIRECTLY INTO PSUM
    (biases folded: layer 0 via a ones-row appended to x^T, layer 1 via a
    K=1 ones matmul). The per-step recurrent matmuls then ACCUMULATE onto
    u in place (start=False), eliminating the per-step DVE add.
  - Per-step chain is just: 16 matmuls -> ScalarE tanh (PSUM->SBUF) ->
    DVE scalar_tensor_tensor state update (contiguous fp16, 2x mode).
  - State stored scaled: htil = h / 0.9 (0.9 folds into weights; final
    0.9 rescale on host). Matmul operands fp16; PSUM/accum fp32.
  - PSUM budget: 2 banks per chunk-buffer x 2 buffers x 2 layers = 8.
  - Layer 1 runs one chunk (64 steps) behind layer 0, interleaved.
"""
import sys
import types

import numpy as np

B_TOT, T, I, U = 32, 1024, 64, 512
NCORES = 8
B = B_TOT // NCORES          # 4 samples per core
ALPHA = 0.9
S = 4 * B                    # 16 state cols per step (4 j-tiles x B)

_COMPILED = {}


# ---------------------------------------------------------------------------
# environment patches (inlined so kernel.py is self-contained)
# ---------------------------------------------------------------------------
def _apply_patches():
    import concourse.tile as tilemod
    from concourse.vector_clock import ScopedClock

    if not getattr(tilemod.TileContext, "_drain_patch_applied", False):
        def _drain_and_barrier(self, tick_clock, wait_clock):
            nc = self.nc
            drain_inst = nc.sync.drain()
            wait_clock.add_sem_waits(
                drain_inst.ins, ScopedClock({None: tick_clock.global_clock})
            )
            waits = list(drain_inst.ins.sync_info.on_wait)
            if len(waits) > 1:
                drain_inst.ins.sync_info.on_wait = waits[:1]
                for w in waits[1:]:
                    extra = nc.sync.drain()
                    si = extra.ins.sync_info
                    if si is None:
                        import bass_rust
                        extra.ins.sync_info = bass_rust.SyncInfo(
                            on_wait=[w], on_update=[]
                        )
                    else:
                        si.on_wait = [w]
            nc.all_engine_barrier()
            assert self.sems is not None
            popped = nc._tile_sem_poison_stack.pop()
            assert popped is self._sem_poison
            nc.clear_and_free_semaphores(list(self.sems.allocated().values()))
            nc.all_engine_barrier()

        tilemod.TileContext._drain_and_barrier = _drain_and_barrier
        tilemod.TileContext._drain_patch_applied = True

    import antenv
    if not hasattr(antenv, "axon_hooks"):
        mod = types.ModuleType("antenv.axon_hooks")
        mod._hook = None
        mod.set_axon_ntff_profile_hook = lambda h: setattr(mod, "_hook", h)
        mod.get_axon_ntff_profile_hook = lambda: mod._hook
        sys.modules["antenv.axon_hooks"] = mod
        antenv.axon_hooks = mod
        try:
            from trn_agent_boot.trn_boot import _ntff_profile_via_ctypes
            hook = _ntff_profile_via_ctypes("/opt/axon/libaxon_pjrt.so")
            if hook is not None:
                mod.set_axon_ntff_profile_hook(hook)
        except Exception:
            pass


def _split_sync_waits(nc, max_waits=1):
    """The public walrus rejects instructions with >2 sync-wait commands.
    Spread overflow waits onto same-engine NOPs inserted just before."""
    import concourse.mybir as mybir

    for f in nc.m.functions:
        for blk in f.blocks:
            insts = blk.instructions
            out = []
            changed = False
            for inst in insts:
                si = getattr(inst, "sync_info", None)
                waits = list(si.on_wait) if si is not None else []
                if len(waits) > max_waits:
                    changed = True
                    overflow = waits[:-max_waits]
                    si.on_wait = waits[-max_waits:]
                    for i in range(0, len(overflow), max_waits):
                        nop = mybir.InstNoOp(
                            name=nc.get_next_instruction_name(),
                            sync_info=mybir.SyncInfo(
                                on_wait=overflow[i:i + max_waits], on_update=[]
                            ),
                            bass_nofuse=True,
                            engine=inst.engine,
                        )
                        out.append(nop)
                out.append(inst)
            if changed:
                blk.instructions = out
    return nc


# ---------------------------------------------------------------------------
# kernel builder
# ---------------------------------------------------------------------------
def build_nc(T=T, Tc=64):
    import contextlib

    import concourse.bass as bass
    import concourse.tile as tile
    import concourse.mybir as mybir

    f32 = mybir.dt.float32
    f16 = mybir.dt.float16
    NC = T // Tc               # number of chunks
    HALF = Tc // 2             # steps per PSUM bank (32)

    nc = bass.Bass(trn_type="TRN2")

    xT_d = nc.declare_dram_parameter("xT", (I + 1, T * B), f16, isOutput=False)
    k0_d = nc.declare_dram_parameter("k0", (I + 1, U), f16, isOutput=False)
    w0_d = nc.declare_dram_parameter("w0", (128, 4 * U), f16, isOutput=False)
    k1_d = nc.declare_dram_parameter("k1", (128, 4 * U), f16, isOutput=False)
    w1_d = nc.declare_dram_parameter("w1", (128, 4 * U), f16, isOutput=False)
    b1_d = nc.declare_dram_parameter("b1", (1, U), f16, isOutput=False)
    h0_d = nc.declare_dram_parameter("h0T", (128, T * S), f16, isOutput=True)
    h1_d = nc.declare_dram_parameter("h1T", (128, T * S), f16, isOutput=True)

    with tile.TileContext(nc) as tc:
        with contextlib.ExitStack() as ctx:
            const = ctx.enter_context(tc.tile_pool(name="const", bufs=1))
            ypool = ctx.enter_context(tc.tile_pool(name="ypool", bufs=3))
            pspool = ctx.enter_context(
                tc.tile_pool(name="pspool", bufs=1, space="PSUM"))

            # --- resident constants -----------------------------------------
            xT = const.tile([I + 1, T * B], f16, tag="xT")
            nc.sync.dma_start(xT[:], xT_d[:, :])
            k0 = const.tile([I + 1, U], f16, tag="k0")
            nc.sync.dma_start(k0[:], k0_d[:, :])
            w0 = const.tile([128, 4 * U], f16, tag="w0")
            nc.sync.dma_start(w0[:], w0_d[:, :])
            k1 = const.tile([128, 4 * U], f16, tag="k1")
            nc.sync.dma_start(k1[:], k1_d[:, :])
            w1 = const.tile([128, 4 * U], f16, tag="w1")
            nc.sync.dma_start(w1[:], w1_d[:, :])
            b1 = const.tile([1, U], f16, tag="b1")
            nc.sync.dma_start(b1[:], b1_d[:, :])
            ones = const.tile([1, 128], f16, tag="ones")
            nc.vector.memset(ones[:], 1.0)
            zero = const.tile([128, S], f16, tag="zero")
            nc.vector.memset(zero[:], 0.0)

            # state chunk buffers, step-major: col = r*16 + j*4 + b
            hist0 = [const.tile([128, Tc * S], f16, tag=f"hist0_{i}",
                                name=f"hist0_{i}") for i in range(2)]
            hist1 = [const.tile([128, Tc * S], f16, tag=f"hist1_{i}",
                                name=f"hist1_{i}") for i in range(2)]

            # PSUM: z/u chunks. One tile per bank (8 banks): [buffer][half].
            # Single-bank tiles keep the dependency tracker from coupling
            # the two halves of a chunk (and GEMM writes vs ACT reads).
            ps0 = [[pspool.tile([128, HALF * S], f32, tag=f"ps0_{i}{h}",
                                name=f"ps0_{i}{h}") for h in range(2)]
                   for i in range(2)]
            ps1 = [[pspool.tile([128, HALF * S], f32, tag=f"ps1_{i}{h}",
                                name=f"ps1_{i}{h}") for h in range(2)]
                   for i in range(2)]

            def ps_out_ap(psb, half, m):
                """Strided PSUM out [128, HALF steps, B] for feature-tile m."""
                v = psb[half][:, :].rearrange("p (t s) -> p t s", s=S)
                return v[:, :, m * B:(m + 1) * B]

            def u0_gemm_thunks(c):
                """u0 chunk c = x^T chunk @ K0 (+bias via ones row), to PSUM.
                Returns one closure per matmul for slot-interleaved emission."""
                ps = ps0[c % 2]

                def mk(half, m):
                    cols = slice((c * Tc + half * HALF) * B,
                                 (c * Tc + (half + 1) * HALF) * B)

                    def emit():
                        nc.tensor.matmul(
                            ps_out_ap(ps, half, m),
                            lhsT=k0[:, m * 128:(m + 1) * 128],
                            rhs=xT[:, cols],
                            start=(m == 0), stop=True,
                            skip_group_check=True)
                    return emit
                return [mk(half, m) for half in range(2) for m in range(4)]

            def u1_half_thunks(c, half):
                """u1 chunk c, psum bank `half`, from hist0 chunk c steps
                [half*32, half*32+32). b1 added via K=1 ones matmul."""
                ps = ps1[c % 2]
                src = hist0[c % 2][:, :].rearrange("p (t s) -> p t s", s=S)
                tsl = slice(half * HALF, (half + 1) * HALF)
                thunks = []

                def mk_ones(m):
                    def emit():
                        nc.tensor.matmul(
                            ps_out_ap(ps, half, m),
                            lhsT=b1[:, m * 128:(m + 1) * 128],
                            rhs=ones[:, :],
                            start=(m == 0), stop=False,
                            skip_group_check=True)
                    return emit

                def mk_k(m, k):
                    def emit():
                        nc.tensor.matmul(
                            ps_out_ap(ps, half, m),
                            lhsT=k1[:, k * U + m * 128: k * U + (m + 1) * 128],
                            rhs=src[:, tsl, k * B:(k + 1) * B],
                            start=False, stop=(k == 3),
                            skip_group_check=True)
                    return emit

                for m in range(4):
                    thunks.append(mk_ones(m))
                    for k in range(4):
                        thunks.append(mk_k(m, k))
                return thunks

            def step(layer, t):
                w, psb, hist = ((w0, ps0, hist0) if layer == 0
                                else (w1, ps1, hist1))
                r = t % Tc
                ps = psb[(t // Tc) % 2][r // HALF]
                rb = r % HALF
                if t == 0:
                    prev = zero[:, :]
                else:
                    prev = hist[((t - 1) // Tc) % 2][
                        :, ((t - 1) % Tc) * S:(((t - 1) % Tc) + 1) * S]
                for m in range(4):
                    out = ps[:, rb * S + m * B: rb * S + (m + 1) * B]
                    for k in range(4):
                        nc.tensor.matmul(
                            out,
                            lhsT=w[:, k * U + m * 128: k * U + (m + 1) * 128],
                            rhs=prev[:, k * B:(k + 1) * B],
                            start=False, stop=(k == 3),
                            skip_group_check=True)
                y = ypool.tile([128, S], f16, tag=f"y{layer}", name=f"y{layer}")
                nc.scalar.activation(
                    y[:], ps[:, rb * S:(rb + 1) * S],
                    mybir.ActivationFunctionType.Tanh)
                nc.vector.scalar_tensor_tensor(
                    hist[(t // Tc) % 2][:, r * S:(r + 1) * S],
                    prev, 0.1, y[:],
                    op0=mybir.AluOpType.mult, op1=mybir.AluOpType.add)

            def dma_out(hist_bufs, dram, c):
                nc.sync.dma_start(
                    dram[:, c * Tc * S:(c + 1) * Tc * S],
                    hist_bufs[c % 2][:, :])

            for th in u0_gemm_thunks(0):
                th()
            for c in range(NC + 1):
                # GEMM matmuls interleaved one-per-slot into the step slots
                # (fills PE idle in the tanh/state-update tail, avoids the
                # end-of-chunk burst that stalls both recurrence chains).
                extra = {}

                def put(r0, thunks):
                    for i, th in enumerate(thunks):
                        extra.setdefault(r0 + i, []).append(th)
                if c >= 1:
                    # u1 second half of chunk c-1: needs L0 steps 32-63 of
                    # chunk c-1 (done); L1 consumes its bank from slot 32.
                    put(2, u1_half_thunks(c - 1, 1))
                if c < NC:
                    # u1 first half of chunk c: needs L0 steps 0-31 (slot 31).
                    put(33, u1_half_thunks(c, 0))
                if c + 1 < NC:
                    put(40, u0_gemm_thunks(c + 1))
                for r in range(Tc):
                    if c < NC:
                        step(0, c * Tc + r)
                    if c >= 1:
                        step(1, (c - 1) * Tc + r)
                    for th in extra.get(r, ()):
                        th()
                if c < NC:
                    dma_out(hist0, h0_d, c)
                if c >= 1:
                    dma_out(hist1, h1_d, c - 1)

    _split_sync_waits(nc)
    return nc


# ---------------------------------------------------------------------------
# host wrapper
# ---------------------------------------------------------------------------
def _prep_weight(w, scale):
    """[U,U] -> [128, 4*U] fp16 with block (k,m) at cols k*U + m*128."""
    a = (scale * w).astype(np.float16)
    return np.ascontiguousarray(
        a.reshape(4, 128, 4, 128).transpose(1, 0, 2, 3).reshape(128, 4 * U))


def _make_in_maps(x, kernel0, rec0, bias0, kernel1, rec1, bias1):
    k0 = np.vstack([kernel0, bias0[None, :]]).astype(np.float16)
    w0 = _prep_weight(rec0, ALPHA)
    k1 = _prep_weight(kernel1, ALPHA)
    w1 = _prep_weight(rec1, ALPHA)
    b1 = bias1[None, :].astype(np.float16)
    in_maps = []
    for c in range(NCORES):
        xc = x[c * B:(c + 1) * B]                       # [B, T, I]
        xT = np.empty((I + 1, T * B), np.float16)
        xT[:I] = xc.transpose(2, 1, 0).reshape(I, T * B)
        xT[I] = 1.0
        in_maps.append({
            "xT": np.ascontiguousarray(xT), "k0": k0, "w0": w0,
            "k1": k1, "w1": w1, "b1": b1,
        })
    return in_maps


def kernel(x, kernel0, rec0, bias0, kernel1, rec1, bias1):
    _apply_patches()
    from concourse.bass_utils import run_bass_kernel_spmd

    x = np.asarray(x, dtype=np.float32)
    kernel0 = np.asarray(kernel0, dtype=np.float32)
    rec0 = np.asarray(rec0, dtype=np.float32)
    bias0 = np.asarray(bias0, dtype=np.float32)
    kernel1 = np.asarray(kernel1, dtype=np.float32)
    rec1 = np.asarray(rec1, dtype=np.float32)
    bias1 = np.asarray(bias1, dtype=np.float32)

    if "nc" not in _COMPILED:
        _COMPILED["nc"] = build_nc()
    nc = _COMPILED["nc"]

    in_maps = _make_in_maps(x, kernel0, rec0, bias0, kernel1, rec1, bias1)
    res = run_bass_kernel_spmd(nc, in_maps, list(range(NCORES)))

    out = np.empty((B_TOT, T, 2 * U), dtype=np.float32)
    for c in range(NCORES):
        for name, off in (("h0T", 0), ("h1T", U)):
            h = res.results[c][name].astype(np.float32) * ALPHA  # [128, T*S]
            # col = t*16 + j*4 + b; feature = j*128 + p
            h = h.reshape(128, T, 4, B).transpose(3, 1, 2, 0)    # [b,t,j,p]
            out[c * B:(c + 1) * B, :, off:off + U] = h.reshape(B, T, U)
    return out


def run_timed(x, kernel0, rec0, bias0, kernel1, rec1, bias1, tmpdir=None):
    """Run with NTFF profiling; returns BassKernelResults with exec_time_ns."""
    _apply_patches()
    import tempfile
    if tmpdir is None:
        tmpdir = tempfile.mkdtemp(prefix="dr_trace_")
    from concourse.bass_utils import run_bass_kernel_spmd
    if "nc" not in _COMPILED:
        _COMPILED["nc"] = build_nc()
    in_maps = _make_in_maps(
        np.asarray(x, np.float32), np.asarray(kernel0, np.float32),
        np.asarray(rec0, np.float32), np.asarray(bias0, np.float32),
        np.asarray(kernel1, np.float32), np.asarray(rec1, np.float32),
        np.asarray(bias1, np.float32))
    return run_bass_kernel_spmd(
        _COMPILED["nc"], in_maps, list(range(NCORES)), trace=True,
        tmpdir=tmpdir)


# revision 11
# speedup vs baseline: 1.0035x; 1.0035x over previous
"""DeepReservoir (2-layer leaky ESN) Trainium2 kernel, v5.

Reference computation (per layer):
    u = x @ K + b
    h_t = 0.1*h_{t-1} + 0.9*tanh(u_t + h_{t-1} @ W)
Layer 1 consumes layer 0's states; output = concat(s0, s1) on features.

Kernel strategy (data-parallel over batch, 8 cores x B=4 samples).

z-recurrence with y-state (y = tanh(z)), which removes the leaky state
update from the critical chain:
    z_{t+1} = [u_{t+1} - 0.1 u_t] + 0.1 z_t + (0.9 W)^T y_t
  - utilde = u_{t+1} - 0.1 u_t is precomputed per chunk by GEMMs directly
    into PSUM (layer 0 from host-prepared xtilde = x_t - 0.1 x_{t-1};
    layer 1 from y0 via h0_t - 0.1 h0_{t-1} = 0.9 y0_t).
  - The per-step matmul burst consumes y_{t-1} straight from the tanh:
    critical chain is burst -> tanh -> burst.  The 0.1*z_t blend is a
    DVE op (psum_{t+1} += 0.1 psum_t) running concurrently with the tanh
    on the other engine; consecutive steps live in opposite PSUM banks
    (even/odd step parity) so writers never collide with readers.
  - The leaky state htil_t = 0.1 htil_{t-1} + y_t (htil = h/0.9) is
    materialized by an off-chain DVE op and DMA'd out as the result.
  - PSUM: per layer per chunk-buffer: even bank + odd bank; x2 buffers
    x2 layers = 8 banks.  Layer 1 runs one chunk (64 steps) behind.
  - All matmul operands fp16; PSUM fp32; weights carry the 0.9 factor;
    final 0.9 rescale on host.
"""
import sys
import types

import numpy as np

B_TOT, T, I, U = 32, 1024, 64, 512
NCORES = 8
B = B_TOT // NCORES          # 4 samples per core
ALPHA = 0.9
S = 4 * B                    # 16 state cols per step (4 j-tiles x B)

_COMPILED = {}


# ---------------------------------------------------------------------------
# environment patches (inlined so kernel.py is self-contained)
# ---------------------------------------------------------------------------
def _apply_patches():
    import concourse.tile as tilemod
    from concourse.vector_clock import ScopedClock

    if not getattr(tilemod.TileContext, "_drain_patch_applied", False):
        def _drain_and_barrier(self, tick_clock, wait_clock):
            nc = self.nc
            drain_inst = nc.sync.drain()
            wait_clock.add_sem_waits(
                drain_inst.ins, ScopedClock({None: tick_clock.global_clock})
            )
            waits = list(drain_inst.ins.sync_info.on_wait)
            if len(waits) > 1:
                drain_inst.ins.sync_info.on_wait = waits[:1]
                for w in waits[1:]:
                    extra = nc.sync.drain()
                    si = extra.ins.sync_info
                    if si is None:
                        import bass_rust
                        extra.ins.sync_info = bass_rust.SyncInfo(
                            on_wait=[w], on_update=[]
                        )
                    else:
                        si.on_wait = [w]
            nc.all_engine_barrier()
            assert self.sems is not None
            popped = nc._tile_sem_poison_stack.pop()
            assert popped is self._sem_poison
            nc.clear_and_free_semaphores(list(self.sems.allocated().values()))
            nc.all_engine_barrier()

        tilemod.TileContext._drain_and_barrier = _drain_and_barrier
        tilemod.TileContext._drain_patch_applied = True

    import antenv
    if not hasattr(antenv, "axon_hooks"):
        mod = types.ModuleType("antenv.axon_hooks")
        mod._hook = None
        mod.set_axon_ntff_profile_hook = lambda h: setattr(mod, "_hook", h)
        mod.get_axon_ntff_profile_hook = lambda: mod._hook
        sys.modules["antenv.axon_hooks"] = mod
        antenv.axon_hooks = mod
        try:
            from trn_agent_boot.trn_boot import _ntff_profile_via_ctypes
            hook = _ntff_profile_via_ctypes("/opt/axon/libaxon_pjrt.so")
            if hook is not None:
                mod.set_axon_ntff_profile_hook(hook)
        except Exception:
            pass


def _split_sync_waits(nc, max_waits=1):
    """The public walrus rejects instructions with >2 sync-wait commands.
    Spread overflow waits onto same-engine NOPs inserted just before."""
    import concourse.mybir as mybir

    for f in nc.m.functions:
        for blk in f.blocks:
            insts = blk.instructions
            out = []
            changed = False
            for inst in insts:
                si = getattr(inst, "sync_info", None)
                waits = list(si.on_wait) if si is not None else []
                if len(waits) > max_waits:
                    changed = True
                    overflow = waits[:-max_waits]
                    si.on_wait = waits[-max_waits:]
                    for i in range(0, len(overflow), max_waits):
                        nop = mybir.InstNoOp(
                            name=nc.get_next_instruction_name(),
                            sync_info=mybir.SyncInfo(
                                on_wait=overflow[i:i + max_waits], on_update=[]
                            ),
                            bass_nofuse=True,
                            engine=inst.engine,
                        )
                        out.append(nop)
                out.append(inst)
            if changed:
                blk.instructions = out
    return nc


# ---------------------------------------------------------------------------
# kernel builder
# ---------------------------------------------------------------------------
def build_nc(T=T, Tc=64):
    import contextlib

    import concourse.bass as bass
    import concourse.tile as tile
    import concourse.mybir as mybir

    f32 = mybir.dt.float32
    f16 = mybir.dt.float16
    NC = T // Tc               # number of chunks
    HALF = Tc // 2             # steps per PSUM bank / parity block (32)

    nc = bass.Bass(trn_type="TRN2")

    xT_d = nc.declare_dram_parameter("xT", (I + 1, T * B), f16, isOutput=False)
    k0_d = nc.declare_dram_parameter("k0", (I + 1, U), f16, isOutput=False)
    w0_d = nc.declare_dram_parameter("w0", (128, 4 * U), f16, isOutput=False)
    k1_d = nc.declare_dram_parameter("k1", (128, 4 * U), f16, isOutput=False)
    w1_d = nc.declare_dram_parameter("w1", (128, 4 * U), f16, isOutput=False)
    b1_d = nc.declare_dram_parameter("b1", (1, U), f16, isOutput=False)
    h0_d = nc.declare_dram_parameter("h0T", (128, T * S), f16, isOutput=True)
    h1_d = nc.declare_dram_parameter("h1T", (128, T * S), f16, isOutput=True)

    with tile.TileContext(nc) as tc:
        with contextlib.ExitStack() as ctx:
            const = ctx.enter_context(tc.tile_pool(name="const", bufs=1))
            pspool = ctx.enter_context(
                tc.tile_pool(name="pspool", bufs=1, space="PSUM"))

            # --- resident constants -----------------------------------------
            xT = const.tile([I + 1, T * B], f16, tag="xT")
            nc.sync.dma_start(xT[:], xT_d[:, :])
            k0 = const.tile([I + 1, U], f16, tag="k0")
            nc.sync.dma_start(k0[:], k0_d[:, :])
            w0 = const.tile([128, 4 * U], f16, tag="w0")
            nc.sync.dma_start(w0[:], w0_d[:, :])
            k1 = const.tile([128, 4 * U], f16, tag="k1")
            nc.sync.dma_start(k1[:], k1_d[:, :])
            w1 = const.tile([128, 4 * U], f16, tag="w1")
            nc.sync.dma_start(w1[:], w1_d[:, :])
            b1 = const.tile([1, U], f16, tag="b1")
            nc.sync.dma_start(b1[:], b1_d[:, :])
            # rhs vectors for the layer-1 bias matmuls: utilde carries
            # 0.9*b1 for t>=1 but 1.0*b1 for t=0 (first 4 cols of the
            # chunk-0 even bank).
            ones9 = const.tile([1, 128], f16, tag="ones9")
            nc.vector.memset(ones9[:], ALPHA)
            ones0 = const.tile([1, 128], f16, tag="ones0")
            nc.vector.memset(ones0[:], ALPHA)
            nc.vector.memset(ones0[:, 0:B], 1.0)
            zero = const.tile([128, S], f16, tag="zero")
            nc.vector.memset(zero[:], 0.0)

            # y chunk buffers, parity-major: y_t at col (t%2)*512+(r//2)*16
            yb0 = [const.tile([128, Tc * S], f16, tag=f"yb0_{i}",
                              name=f"yb0_{i}") for i in range(2)]
            yb1 = [const.tile([128, Tc * S], f16, tag=f"yb1_{i}",
                              name=f"yb1_{i}") for i in range(2)]
            # htil chunk buffers (output), step-major: col r*16
            hist0 = [const.tile([128, Tc * S], f16, tag=f"hist0_{i}",
                                name=f"hist0_{i}") for i in range(2)]
            hist1 = [const.tile([128, Tc * S], f16, tag=f"hist1_{i}",
                                name=f"hist1_{i}") for i in range(2)]

            # PSUM: z/u slots. [buffer][parity] single-bank tiles; 8 banks.
            ps0 = [[pspool.tile([128, HALF * S], f32, tag=f"ps0_{i}{h}",
                                name=f"ps0_{i}{h}") for h in range(2)]
                   for i in range(2)]
            ps1 = [[pspool.tile([128, HALF * S], f32, tag=f"ps1_{i}{h}",
                                name=f"ps1_{i}{h}") for h in range(2)]
                   for i in range(2)]

            def ps_slot(psb, t):
                """[128,16] PSUM slice for step t (chunk-local bank pos)."""
                r = t % Tc
                return psb[(t // Tc) % 2][r % 2][
                    :, (r // 2) * S:(r // 2 + 1) * S]

            def y_slot(ybufs, t):
                r = t % Tc
                col = ((r % 2) * HALF + r // 2) * S
                return ybufs[(t // Tc) % 2][:, col:col + S]

            def u0_gemm_thunks(c):
                """utilde0 chunk c from host-prepared xtilde^T, to PSUM.
                One closure per matmul for slot-interleaved emission."""
                psb = ps0[c % 2]

                def mk(par, m):
                    cols = slice((c * Tc + par * HALF) * B,
                                 (c * Tc + (par + 1) * HALF) * B)

                    def emit():
                        v = psb[par][:, :].rearrange("p (t s) -> p t s", s=S)
                        nc.tensor.matmul(
                            v[:, :, m * B:(m + 1) * B],
                            lhsT=k0[:, m * 128:(m + 1) * 128],
                            rhs=xT[:, cols],
                            start=(m == 0), stop=True,
                            skip_group_check=True)
                    return emit
                return [mk(par, m) for par in range(2) for m in range(4)]

            def u1_gemm_thunks(c, part):
                """utilde1 chunk c = 0.9*y0@K1 (+(0.9|1.0)*b1 via ones MM).
                part 0: bias matmuls + bank positions 0..15 (steps 0..31),
                emittable once L0 step c*64+31 is done.  part 1: positions
                16..31 (steps 32..63), emittable once L0 chunk c is done."""
                psb = ps1[c % 2]
                src = yb0[c % 2][:, :].rearrange("p (t s) -> p t s", s=S)
                thunks = []

                def mk_ones(par, m):
                    def emit():
                        v = psb[par][:, :].rearrange("p (t s) -> p t s", s=S)
                        ones = ones0 if (c == 0 and par == 0) else ones9
                        nc.tensor.matmul(
                            v[:, :, m * B:(m + 1) * B],
                            lhsT=b1[:, m * 128:(m + 1) * 128],
                            rhs=ones[:, :],
                            start=(m == 0), stop=False,
                            skip_group_check=True)
                    return emit

                def mk_k(par, pos0, m, k):
                    def emit():
                        v = psb[par][:, :].rearrange("p (t s) -> p t s", s=S)
                        nc.tensor.matmul(
                            v[:, pos0:pos0 + HALF // 2, m * B:(m + 1) * B],
                            lhsT=k1[:, k * U + m * 128: k * U + (m + 1) * 128],
                            rhs=src[:, par * HALF + pos0:
                                    par * HALF + pos0 + HALF // 2,
                                    k * B:(k + 1) * B],
                            start=False, stop=(k == 3),
                            skip_group_check=True)
                    return emit

                if part == 0:
                    for par in range(2):
                        for m in range(4):
                            thunks.append(mk_ones(par, m))
                    for par in range(2):
                        for m in range(4):
                            for k in range(4):
                                thunks.append(mk_k(par, 0, m, k))
                else:
                    for par in range(2):
                        for m in range(4):
                            for k in range(4):
                                thunks.append(mk_k(par, HALF // 2, m, k))
                return thunks

            spool = ctx.enter_context(tc.tile_pool(name="spool", bufs=3))

            def step(layer, t):
                w, psb, ybufs = (
                    (w0, ps0, yb0) if layer == 0 else (w1, ps1, yb1))
                cur = ps_slot(psb, t)
                if t > 0:
                    yprev = y_slot(ybufs, t - 1)
                    for m in range(4):
                        out = cur[:, m * B:(m + 1) * B]
                        for k in range(4):
                            nc.tensor.matmul(
                                out,
                                lhsT=w[:, k * U + m * 128:
                                       k * U + (m + 1) * 128],
                                rhs=yprev[:, k * B:(k + 1) * B],
                                start=False, stop=(k == 3),
                                skip_group_check=True)
                # 0.1*z_t feeds z_{t+1}; two DVE ops (an instruction may
                # read only one PSUM operand) running concurrently with the
                # tanh; consecutive steps live in opposite psum banks.
                if t < T - 1:
                    s = spool.tile([128, S], f32, tag=f"s{layer}",
                                   name=f"s{layer}")
                    nc.vector.tensor_scalar_mul(s[:], cur, 1.0 - ALPHA)
                    nxt = ps_slot(psb, t + 1)
                    nc.vector.tensor_add(nxt, nxt, s[:])
                y = y_slot(ybufs, t)
                nc.scalar.activation(
                    y, cur, mybir.ActivationFunctionType.Tanh)

            def stt_step(layer, t):
                """Off-chain leaky state htil_t = 0.1 htil_{t-1} + y_t."""
                ybufs, hist = ((yb0, hist0) if layer == 0 else (yb1, hist1))
                r = t % Tc
                prev = (zero[:, :] if t == 0 else
                        hist[((t - 1) // Tc) % 2][
                            :, ((t - 1) % Tc) * S:(((t - 1) % Tc) + 1) * S])
                nc.vector.scalar_tensor_tensor(
                    hist[(t // Tc) % 2][:, r * S:(r + 1) * S],
                    prev, 1.0 - ALPHA, y_slot(ybufs, t),
                    op0=mybir.AluOpType.mult, op1=mybir.AluOpType.add)

            def dma_out(hist_bufs, dram, c):
                nc.sync.dma_start(
                    dram[:, c * Tc * S:(c + 1) * Tc * S],
                    hist_bufs[c % 2][:, :])

            for th in u0_gemm_thunks(0):
                th()
            for c in range(NC + 1):
                extra = {}

                def put(r0, thunks):
                    for i, th in enumerate(thunks):
                        extra.setdefault(r0 + i, []).append(th)
                if c >= 1:
                    put(1, u1_gemm_thunks(c - 1, 1))
                if c < NC:
                    # 40 thunks, 2 per slot over r=33..52 (order preserved:
                    # the bias matmuls open each bank before its k matmuls)
                    th0 = u1_gemm_thunks(c, 0)
                    put(33, th0[0::2])
                    put(33, th0[1::2])
                if c + 1 < NC:
                    put(40, u0_gemm_thunks(c + 1))
                for r in range(Tc):
                    if c < NC:
                        step(0, c * Tc + r)
                    if c >= 1:
                        step(1, (c - 1) * Tc + r)
                    # state updates last: they gate nothing on the critical
                    # chain, so keep them out of the DVE queue's way.
                    if c < NC:
                        stt_step(0, c * Tc + r)
                    if c >= 1:
                        stt_step(1, (c - 1) * Tc + r)
                    for th in extra.get(r, ()):
                        th()
                if c < NC:
                    dma_out(hist0, h0_d, c)
                if c >= 1:
                    dma_out(hist1, h1_d, c - 1)

    _split_sync_waits(nc)
    return nc


# ---------------------------------------------------------------------------
# host wrapper
# ---------------------------------------------------------------------------
def _prep_weight(w, scale):
    """[U,U] -> [128, 4*U] fp16 with block (k,m) at cols k*U + m*128."""
    a = (scale * w).astype(np.float16)
    return np.ascontiguousarray(
        a.reshape(4, 128, 4, 128).transpose(1, 0, 2, 3).reshape(128, 4 * U))


def _make_in_maps(x, kernel0, rec0, bias0, kernel1, rec1, bias1):
    k0 = np.vstack([kernel0, bias0[None, :]]).astype(np.float16)
    w0 = _prep_weight(rec0, ALPHA)
    k1 = _prep_weight(kernel1, ALPHA)
    w1 = _prep_weight(rec1, ALPHA)
    b1 = bias1[None, :].astype(np.float16)
    # xtilde_t = x_t - 0.1 x_{t-1} (t>=1), x_0 at t=0; the ones row is the
    # bias multiplier (0.9 for t>=1, 1.0 at t=0).
    in_maps = []
    for c in range(NCORES):
        xc = np.asarray(x[c * B:(c + 1) * B], np.float32)  # [B, T, I]
        xt = xc.copy()
        xt[:, 1:] -= (1.0 - ALPHA) * xc[:, :-1]
        full = np.empty((I + 1, T, B), np.float32)
        full[:I] = xt.transpose(2, 1, 0)
        full[I] = ALPHA
        full[I, 0] = 1.0
        # chunk-parity-major column order: per chunk, even steps then odd.
        v = full.reshape(I + 1, T // 64, 32, 2, B)
        v = v.transpose(0, 1, 3, 2, 4)          # [I+1, NC, par, pos, B]
        xT = np.ascontiguousarray(v.reshape(I + 1, T * B)).astype(np.float16)
        in_maps.append({
            "xT": xT, "k0": k0, "w0": w0, "k1": k1, "w1": w1, "b1": b1,
        })
    return in_maps


def kernel(x, kernel0, rec0, bias0, kernel1, rec1, bias1):
    _apply_patches()
    from concourse.bass_utils import run_bass_kernel_spmd

    x = np.asarray(x, dtype=np.float32)
    kernel0 = np.asarray(kernel0, dtype=np.float32)
    rec0 = np.asarray(rec0, dtype=np.float32)
    bias0 = np.asarray(bias0, dtype=np.float32)
    kernel1 = np.asarray(kernel1, dtype=np.float32)
    rec1 = np.asarray(rec1, dtype=np.float32)
    bias1 = np.asarray(bias1, dtype=np.float32)

    if "nc" not in _COMPILED:
        _COMPILED["nc"] = build_nc()
    nc = _COMPILED["nc"]

    in_maps = _make_in_maps(x, kernel0, rec0, bias0, kernel1, rec1, bias1)
    res = run_bass_kernel_spmd(nc, in_maps, list(range(NCORES)))

    out = np.empty((B_TOT, T, 2 * U), dtype=np.float32)
    for c in range(NCORES):
        for name, off in (("h0T", 0), ("h1T", U)):
            h = res.results[c][name].astype(np.float32) * ALPHA  # [128, T*S]
            # col = t*16 + j*4 + b; feature = j*128 + p
            h = h.reshape(128, T, 4, B).transpose(3, 1, 2, 0)    # [b,t,j,p]
            out[c * B:(c + 1) * B, :, off:off + U] = h.reshape(B, T, U)
    return out


def run_timed(x, kernel0, rec0, bias0, kernel1, rec1, bias1, tmpdir=None):
    """Run with NTFF profiling; returns BassKernelResults with exec_time_ns."""
    _apply_patches()
    import tempfile
    if tmpdir is None:
        tmpdir = tempfile.mkdtemp(prefix="dr_trace_")
    from concourse.bass_utils import run_bass_kernel_spmd
    if "nc" not in _COMPILED:
        _COMPILED["nc"] = build_nc()
    in_maps = _make_in_maps(
        np.asarray(x, np.float32), np.asarray(kernel0, np.float32),
        np.asarray(rec0, np.float32), np.asarray(bias0, np.float32),
        np.asarray(kernel1, np.float32), np.asarray(rec1, np.float32),
        np.asarray(bias1, np.float32))
    return run_bass_kernel_spmd(
        _COMPILED["nc"], in_maps, list(range(NCORES)), trace=True,
        tmpdir=tmpdir)
